# revision 83
# baseline (speedup 1.0000x reference)
"""Trainium2 Bass kernel for nn_ClaimEncoder (dense_mlp).

Math (per row):
  feats = [sin/cos point-encoders (2x256), leaky number-encoders (3x128)]  -> [896]
  h   = leaky_relu(feats @ W1 + b1)   -> [512]
  out = leaky_relu(h @ W2 + b2)       -> [512]

Strategy: pure data parallel over 8 NeuronCores (16384 rows each).

Device-side design (per core, encoder super-tiles of SB=1024 batch
columns = 2 matmul tiles of NB=512):

  * PE does ONLY the 44 MLP matmuls per NB tile (bf16 operands, 1
    cycle/row) -> ~9.4us per tile, ~300us/core: the compute roofline
    every other engine must stay under.
  * Encoder, sin path (4 chunks of 128 features): a strided DMA
    partition-broadcasts the 7 value rows (fp16); Pool computes
    z' = w[p]*v + b[p] with (w, b) pre-scaled by 1/2pi; DVE range-
    reduces with the fp32 magic-constant rounding trick (k = round(z'),
    y = k - z'; the DVE mod ALU op is NOT supported by walrus codegen);
    ACT computes sin(-2pi*y) = sin(wx+b), writing bf16 feats directly.
  * Encoder, number path (3 chunks): Pool z = w[p]*v + b[p] (bf16),
    DVE prelu via max(0.01*z, z) -> bf16 feats.
  * L1: stationary = W1 chunk [128f, 128ced], moving = featsT -> hT
    PSUM [128ced, 512b]; ACT eviction fuses +b1 (per-partition bias)
    and leaky into one Prelu op, bf16 h.
  * L2 is emitted k-outer (all 4 output banks accumulate as each h
    chunk lands) with stationary = W2 chunk, moving = h. Output stays
    CED-major so b2 is a per-partition ACT bias: one Prelu eviction
    per bank. outT [CED, BC] bf16; the host transposes back and
    upcasts.
  * Startup: super-tile 0 runs the encoder at NB granularity with z'
    spread across DVE/ACT (Pool handles the number chunks), chunk
    consumption ordered by readiness, and 128-wide PE filler matmuls
    plugging the arrival stutter (also keeps the p-state ramped).
    The tail tile is split into two 256-column slivers with each
    sliver's stores issued right after its own eviction (SP + ACT
    queues), shortening the final evict->store drain.

Engine budgets per core: PE ~306us busy (~300us roofline + warm
fillers), ACT ~205us, Pool ~155us, DVE ~150us, DMA ~134us.
Measured: 315945 ns TimelineSim per core, HW rel err 4.2e-3 vs the
2e-2 gate.
"""

import numpy as np
import ml_dtypes

import concourse.bass as bass
import concourse.tile as tile
import concourse.mybir as mybir
from concourse import bacc
from concourse.bass_utils import run_bass_kernel_spmd

# Problem shapes (hardcoded; kernel.py must be self-contained).
B = 131072
N_CORES = 8
BC = B // N_CORES          # 16384 rows per core
PED = 256
NED = 128
CED = 512
Q = PED // 4               # 64
FEAT = 2 * PED + 3 * NED   # 896
NB = 512                   # batch columns per matmul tile
SB = 1024                  # encoder super-tile batch columns
NSUP = BC // SB            # 16 super-tiles
KC = FEAT // 128           # 7 feature chunks
MC = CED // 128            # 4 output chunks
N_WARM = 34                # upfront PE warm-up matmuls (128-wide)

TWO_PI = 2.0 * np.pi
# fp32 round-to-nearest-integer magic constant: adding it forces the
# mantissa to integer granularity (valid for |x| << 2^22).
MAGIC = 1.5 * 2.0 ** 23

# super-0 chunk consumption order, sorted by expected readiness
ORDER0 = (0, 4, 1, 5, 2, 6, 3)

F32 = mybir.dt.float32
F16 = mybir.dt.float16
BF16 = mybir.dt.bfloat16
AF = mybir.ActivationFunctionType
ALU = mybir.AluOpType


def _build_bass():
    nc = bacc.Bacc(
        "TRN2",
        target_bir_lowering=False,
        debug=False,
        enable_asserts=False,
        num_devices=N_CORES,
    )

    a8 = nc.dram_tensor("a8", [8, BC], F16, kind="ExternalInput").ap()
    # host-packed SBUF layouts: w1s[p, c*512+j] = W1[c*128+p, j] (bf16)
    w1 = nc.dram_tensor("w1", [128, KC * CED], BF16, kind="ExternalInput").ap()
    # w2s[p, k*512+j] = W2[k*128+p, j] (bf16)
    w2 = nc.dram_tensor("w2", [128, MC * CED], BF16, kind="ExternalInput").ap()
    b1c = nc.dram_tensor("b1c", [128, MC], F32, kind="ExternalInput").ap()
    b2c = nc.dram_tensor("b2c", [128, MC], F32, kind="ExternalInput").ap()
    # number-encoder per-feature (w, b) pairs: cols [t_w,t_b,ws_w,ws_b,wd_w,wd_b]
    nwb = nc.dram_tensor("nwb", [128, 6], F32, kind="ExternalInput").ap()
    # point-encoder per-feature (w, b)/2pi pairs
    pwb = nc.dram_tensor("pwb", [128, 8], F32, kind="ExternalInput").ap()
    # CED-major output; host transposes back
    outT = nc.dram_tensor("outT", [CED, BC], BF16, kind="ExternalOutput").ap()

    with tile.TileContext(nc) as tc:
        with (
            tc.tile_pool(name="consts", bufs=1) as consts,
            tc.tile_pool(name="vbp", bufs=2) as vb_pool,
            tc.tile_pool(name="zpp", bufs=4) as zp_pool,
            tc.tile_pool(name="yp", bufs=4) as y_pool,
            tc.tile_pool(name="rrp", bufs=4) as rr_pool,
            tc.tile_pool(name="znp", bufs=6) as zn_pool,
            tc.tile_pool(name="featsp", bufs=2) as feats_pool,
            tc.tile_pool(name="hp", bufs=2) as h_pool,
            tc.tile_pool(name="tmpp", bufs=4) as tmp_pool,
            tc.tile_pool(name="outp", bufs=8) as out_pool,
            tc.tile_pool(name="l1_ps", bufs=4, space="PSUM") as l1_psum,
            tc.tile_pool(name="l2_ps", bufs=4, space="PSUM") as l2_psum,
        ):
            warm = consts.tile([128, 128], BF16)
            nc.vector.memset(warm[:], 0.0)
            magic_t = consts.tile([128, 1], F32)
            nc.vector.memset(magic_t[:], MAGIC)

            consts_sb = {}
            vb_tiles = {}
            feats_tiles = {}
            h_tiles = {}

            def emit_vb_dma(s):
                """Partition-broadcast the 7 value rows for super-tile s."""
                vb = vb_pool.tile([128, KC * SB], F16, name=f"vb_{s}", tag="vb")
                vb_tiles[s] = vb

                def bc(c, off, n):
                    src = bass.AP(
                        tensor=a8.tensor, offset=c * BC + s * SB + off,
                        ap=[[0, 128], [1, n]],
                    )
                    nc.sync.dma_start(out=vb[:, c * SB + off: c * SB + off + n],
                                      in_=src)

                # split halves: first chunks usable at half the latency
                src_a = bass.AP(
                    tensor=a8.tensor, offset=s * SB,
                    ap=[[0, 128], [BC, 4], [1, SB]],
                )
                nc.sync.dma_start(out=vb[:, 0:4 * SB], in_=src_a)
                src_b = bass.AP(
                    tensor=a8.tensor, offset=4 * BC + s * SB,
                    ap=[[0, 128], [BC, 3], [1, SB]],
                )
                nc.sync.dma_start(out=vb[:, 4 * SB:], in_=src_b)
                return vb

            def emit_startup_dmas():
                """Super-0 NB-grained broadcasts in consumption order, with
                consts/weights and super-1's broadcast interleaved so every
                consumer's first input lands just in time."""
                vb0 = vb_pool.tile([128, KC * SB], F16, name="vb_0", tag="vb")
                vb_tiles[0] = vb0
                vb1 = vb_pool.tile([128, KC * SB], F16, name="vb_1", tag="vb")
                vb_tiles[1] = vb1
                w1_t = consts.tile([128, KC * CED], BF16, name="w1s")
                b1_t = consts.tile([128, MC], F32, name="b1c")
                pwb_t = consts.tile([128, 8], F32, name="pwb")
                nwb_t = consts.tile([128, 6], F32, name="nwb")
                b2_t = consts.tile([128, MC], F32, name="b2c")
                w2_t = consts.tile([128, MC * CED], BF16, name="w2s")
                consts_sb.update(w1=w1_t, b1=b1_t, nwb=nwb_t, pwb=pwb_t,
                                 w2=w2_t, b2=b2_t)

                def bc0(c, off):
                    src = bass.AP(
                        tensor=a8.tensor, offset=c * BC + off,
                        ap=[[0, 128], [1, NB]],
                    )
                    nc.sync.dma_start(out=vb0[:, c * SB + off: c * SB + off + NB],
                                      in_=src)

                for i, c in enumerate(ORDER0):
                    bc0(c, 0)
                    if i == 0:
                        nc.sync.dma_start(out=pwb_t[:], in_=pwb[:, :])
                        nc.sync.dma_start(out=nwb_t[:], in_=nwb[:, :])
                        nc.sync.dma_start(out=w1_t[:, 0:4 * CED],
                                          in_=w1[:, 0:4 * CED])
                    elif i == 1:
                        nc.sync.dma_start(out=w1_t[:, 4 * CED:],
                                          in_=w1[:, 4 * CED:])
                    elif i == 2:
                        nc.sync.dma_start(out=b1_t[:], in_=b1c[:, :])
                nc.sync.dma_start(out=b2_t[:], in_=b2c[:, :])
                nc.sync.dma_start(out=w2_t[:], in_=w2[:, :])
                # super-0 second half, then super-1 (its Pool z' chain gates
                # the whole fill phase, so vb1 goes right after the last
                # broadcast Pool consumes first)
                for i, c in enumerate(ORDER0):
                    bc0(c, NB)
                    if i == 5:
                        src_a = bass.AP(
                            tensor=a8.tensor, offset=SB,
                            ap=[[0, 128], [BC, 4], [1, SB]],
                        )
                        nc.sync.dma_start(out=vb1[:, 0:4 * SB], in_=src_a)
                    elif i == 6:
                        src_b = bass.AP(
                            tensor=a8.tensor, offset=4 * BC + SB,
                            ap=[[0, 128], [BC, 3], [1, SB]],
                        )
                        nc.sync.dma_start(out=vb1[:, 4 * SB:], in_=src_b)

            def _enc_sin(s, c, vbc, dst, g, z_eng):
                """z' -> mod-range-reduce -> sin for one sin chunk slice."""
                pwb_t = consts_sb["pwb"]
                wc = pwb_t[:, 2 * c:2 * c + 1]
                bcol = pwb_t[:, 2 * c + 1:2 * c + 2]
                zp = zp_pool.tile([128, g], F32, name=f"zp_{s}_{c}_{g}",
                                  tag=f"zp{g}")
                if z_eng == "dve":
                    nc.vector.tensor_scalar(
                        zp[:], vbc, wc, bcol, op0=ALU.mult, op1=ALU.add)
                elif z_eng == "act":
                    nc.scalar.activation(
                        zp[:], vbc, AF.Identity, scale=wc, bias=bcol)
                else:
                    nc.gpsimd.tensor_scalar(
                        zp[:], vbc, wc, bcol, op0=ALU.mult, op1=ALU.add)
                # fp32 magic-constant range reduction: k = round(z'),
                # y = k - z' in [-0.5, 0.5]; sin(-2pi*y) = sin(w*x+b)
                rr = rr_pool.tile([128, g], F32, name=f"rr_{s}_{c}_{g}",
                                  tag=f"rr{g}")
                if z_eng == "act":
                    nc.scalar.activation(rr[:], zp[:], AF.Identity,
                                         bias=magic_t[:])
                else:
                    nc.vector.tensor_scalar_add(rr[:], zp[:], MAGIC)
                y = y_pool.tile([128, g], F32, name=f"y_{s}_{c}_{g}",
                                tag=f"y{g}")
                nc.vector.scalar_tensor_tensor(
                    y[:], rr[:], MAGIC, zp[:],
                    op0=ALU.subtract, op1=ALU.subtract)
                nc.scalar.activation(dst, y[:], AF.Sin, scale=-TWO_PI)

            def _enc_num(s, c, vbc, dst, g):
                """z -> prelu for one number chunk slice."""
                i = c - 4
                nwb_t = consts_sb["nwb"]
                zn = zn_pool.tile([128, g], BF16, name=f"zn_{s}_{i}_{g}",
                                  tag=f"zn{g}")
                nc.gpsimd.tensor_scalar(
                    zn[:], vbc, nwb_t[:, 2 * i:2 * i + 1],
                    nwb_t[:, 2 * i + 1:2 * i + 2],
                    op0=ALU.mult, op1=ALU.add)
                nc.vector.scalar_tensor_tensor(
                    dst, zn[:], 0.01, zn[:], op0=ALU.mult, op1=ALU.max)

            def emit_enc(s):
                """Encoder compute for super-tile s -> feats (bf16)."""
                vb = vb_tiles.pop(s)
                feats = feats_pool.tile([128, KC * SB], BF16,
                                        name=f"feats_{s}", tag="feats")
                feats_tiles[s] = feats
                if s == 0:
                    # NB-grained, readiness-ordered, z' spread across the
                    # still-idle DVE/ACT
                    for off in (0, NB):
                        for c in ORDER0:
                            vbc = vb[:, c * SB + off: c * SB + off + NB]
                            dst = feats[:, c * SB + off: c * SB + off + NB]
                            if c < 4:
                                _enc_sin(s, c, vbc, dst, NB,
                                         "dve" if c % 2 == 0 else "act")
                            else:
                                _enc_num(s, c, vbc, dst, NB)
                else:
                    # interleave sin/num chunk production to match the MLP's
                    # ORDER0 consumption order (matters in the fill phase
                    # where the Pool z' chain is the critical path); supers
                    # 1-3 offload the first-consumed z' chunks to DVE while
                    # Pool still catches up from the fill transient
                    for c in ORDER0:
                        if c < 4:
                            eng = "dve" if (1 <= s <= 3 and c in (0, 1)) \
                                else "pool"
                            _enc_sin(s, c, vb[:, c * SB:(c + 1) * SB],
                                     feats[:, c * SB:(c + 1) * SB], SB, eng)
                        else:
                            _enc_num(s, c, vb[:, c * SB:(c + 1) * SB],
                                     feats[:, c * SB:(c + 1) * SB], SB)

            warm_ctr = [0]

            def emit_warm(n, pool):
                """128-wide filler matmuls: keep PE busy/ramped in gaps."""
                for _ in range(n):
                    i = warm_ctr[0]
                    warm_ctr[0] += 1
                    wp = pool.tile([128, NB], F32, name=f"warm_{i}",
                                   tag="l1p" if pool is l1_psum else "l2p")
                    nc.tensor.matmul(wp[:, 0:128], warm[:], warm[:],
                                     start=True, stop=True)

            def emit_mlp_nb(s, nb, dve_ev, warm_fill=False, slivers=1,
                            chunk_order=ORDER0):
                """L1+L2+store for matmul tile nb of super-tile s."""
                w1_t = consts_sb["w1"]
                w2_t = consts_sb["w2"]
                b1_t = consts_sb["b1"]
                b2_t = consts_sb["b2"]
                feats = feats_tiles[s]
                h = h_tiles[s]
                sw = NB // slivers
                osbs = [out_pool.tile([128, NB], BF16, name=f"osb_{s}_{nb}_{m}",
                                      tag="osb") for m in range(MC)]
                for sl in range(slivers):
                    col = nb * NB + sl * sw
                    # L1, m-outer: bank m evicts while bank m+1 accumulates
                    for m in range(MC):
                        l1p = l1_psum.tile([128, NB], F32,
                                           name=f"l1p_{s}_{nb}_{m}_{sl}",
                                           tag="l1p")
                        for ci, c in enumerate(chunk_order):
                            nc.tensor.matmul(
                                l1p[:, 0:sw],
                                w1_t[:, c * CED + m * 128:
                                     c * CED + (m + 1) * 128],
                                feats[:, c * SB + col: c * SB + col + sw],
                                start=(ci == 0),
                                stop=(ci == KC - 1),
                            )
                            if warm_fill and m == 0 and ci < 6:
                                emit_warm(4, l2_psum)
                            elif warm_fill and m == 1 and ci < 4:
                                emit_warm(2, l2_psum)
                        nc.scalar.activation(
                            h[:, m * SB + col: m * SB + col + sw],
                            l1p[:, 0:sw],
                            AF.Prelu, bias=b1_t[:, m:m + 1], alpha=0.01)
                    # L2, k-outer: all 4 banks accumulate as h chunks land
                    l2ps = [l2_psum.tile([128, NB], F32,
                                         name=f"l2p_{s}_{nb}_{m}_{sl}",
                                         tag="l2p")
                            for m in range(MC)]
                    for k in range(MC):
                        rhs = h[:, k * SB + col: k * SB + col + sw]
                        for m in range(MC):
                            nc.tensor.matmul(
                                l2ps[m][:, 0:sw],
                                w2_t[:, k * CED + m * 128: k * CED + (m + 1) * 128],
                                rhs,
                                start=(k == 0),
                                stop=(k == MC - 1),
                            )
                    # tail slivers evict the ACT-stored chunks (m2, m3) first
                    # so their store issues overlap the remaining evictions
                    ev_order = (2, 3, 0, 1) if slivers > 1 else range(MC)
                    for m in ev_order:
                        dst = osbs[m][:, sl * sw:(sl + 1) * sw]
                        if m in dve_ev:
                            t = tmp_pool.tile([128, sw], F32,
                                              name=f"t_{s}_{nb}_{m}_{sl}",
                                              tag="t")
                            nc.vector.tensor_scalar_add(t[:], l2ps[m][:, 0:sw],
                                                        b2_t[:, m:m + 1])
                            nc.vector.scalar_tensor_tensor(
                                dst, t[:], 0.01, t[:],
                                op0=ALU.mult, op1=ALU.max)
                        else:
                            nc.scalar.activation(
                                dst, l2ps[m][:, 0:sw], AF.Prelu,
                                bias=b2_t[:, m:m + 1], alpha=0.01)
                        if slivers > 1:
                            # tail: store each sliver chunk immediately after
                            # its own eviction (interleaved, so the ACT-queue
                            # stores are not stuck behind later evictions)
                            bt = s * SB + nb * NB + sl * sw
                            q = (nc.scalar if sl == slivers - 1 and m >= 2
                                 else nc.sync)
                            q.dma_start(
                                out=outT[m * 128:(m + 1) * 128, bt:bt + sw],
                                in_=osbs[m][:, sl * sw:(sl + 1) * sw])
                if slivers == 1:
                    bt = s * SB + nb * NB
                    for m in range(MC):
                        nc.sync.dma_start(
                            out=outT[m * 128:(m + 1) * 128, bt:bt + NB],
                            in_=osbs[m][:])

            def emit_mlp(s):
                h_tiles[s] = h_pool.tile([128, MC * SB], BF16,
                                         name=f"h_{s}", tag="h")
                if s == 0:
                    emit_mlp_nb(s, 0, dve_ev=(), warm_fill=True,
                                chunk_order=ORDER0)
                    emit_mlp_nb(s, 1, dve_ev=(), chunk_order=ORDER0)
                elif s == 1:
                    emit_mlp_nb(s, 0, dve_ev=(), chunk_order=ORDER0)
                    emit_mlp_nb(s, 1, dve_ev=(), chunk_order=ORDER0)
                elif s == NSUP - 1:
                    emit_mlp_nb(s, 0, dve_ev=())
                    emit_mlp_nb(s, 1, dve_ev=(1,), slivers=2)
                else:
                    emit_mlp_nb(s, 0, dve_ev=())
                    emit_mlp_nb(s, 1, dve_ev=())
                del h_tiles[s]
                feats_tiles.pop(s)

            # ---- emission ----
            emit_startup_dmas()
            # PE warm-up: ramp the p-state while the first broadcasts land
            emit_warm(N_WARM, l1_psum)
            emit_enc(0)
            emit_vb_dma(2)
            emit_mlp(0)
            # skew-1 steady state: enc(s+1) right after mlp(s), so encoder
            # ops never head-of-line-block the current super's evictions
            emit_enc(1)
            for s in range(1, NSUP):
                if s + 2 <= NSUP - 1:
                    emit_vb_dma(s + 2)
                emit_mlp(s)
                if s + 1 <= NSUP - 1:
                    emit_enc(s + 1)

    nc.compile()
    return nc


def _host_pack(inputs):
    f32 = lambda k: np.ascontiguousarray(np.asarray(inputs[k], dtype=np.float32))
    src = f32("src_xy")
    dst = f32("dst_xy")

    a8 = np.zeros((8, B), np.float32)
    a8[0] = src[:, 0]
    a8[1] = src[:, 1]
    a8[2] = dst[:, 0]
    a8[3] = dst[:, 1]
    a8[4] = f32("time_s")
    a8[5] = f32("wait_src")
    a8[6] = f32("wait_dst")
    a8 = a8.astype(np.float16)

    # point-encoder (w, b)/2pi per feature: [128, 8], col pair per chunk.
    # chunk c partition p: p<64 -> sin block, p>=64 -> cos block (+pi/2).
    # bias offset +8 periods keeps z' positive for the mod range reduction.
    pwb = np.empty((128, 8), np.float32)
    for c, (pfx, ax) in enumerate((("src", "x"), ("src", "y"),
                                   ("dst", "x"), ("dst", "y"))):
        pwb[:64, 2 * c] = f32(f"{pfx}_ws{ax}") / TWO_PI
        pwb[:64, 2 * c + 1] = f32(f"{pfx}_bs{ax}") / TWO_PI
        pwb[64:, 2 * c] = f32(f"{pfx}_wc{ax}") / TWO_PI
        pwb[64:, 2 * c + 1] = (f32(f"{pfx}_bc{ax}") + np.pi / 2) / TWO_PI
    nwb = np.empty((128, 6), np.float32)
    for i, pfx in enumerate(("t", "ws", "wd")):
        nwb[:, 2 * i] = f32(f"{pfx}_w")
        nwb[:, 2 * i + 1] = f32(f"{pfx}_b")

    # W1 [896, 512] -> [128, 7*512] bf16 chunk-major; W2 [512,512] -> [128, 4*512]
    w1 = f32("W1").reshape(KC, 128, CED).transpose(1, 0, 2).reshape(128, KC * CED)
    w1 = np.ascontiguousarray(w1).astype(ml_dtypes.bfloat16)
    w2 = f32("W2").reshape(MC, 128, CED).transpose(1, 0, 2).reshape(128, MC * CED)
    w2 = np.ascontiguousarray(w2).astype(ml_dtypes.bfloat16)
    # b1c[p, m] = b1[m*128+p]
    b1c = np.ascontiguousarray(f32("b1").reshape(MC, 128).T)
    b2c = np.ascontiguousarray(f32("b2").reshape(MC, 128).T)
    return a8, pwb, nwb, w1, b1c, w2, b2c


def _in_maps(inputs):
    a8, pwb, nwb, w1, b1c, w2, b2c = _host_pack(inputs)
    maps = []
    for i in range(N_CORES):
        maps.append({
            "a8": np.ascontiguousarray(a8[:, i * BC:(i + 1) * BC]),
            "pwb": pwb,
            "w1": w1,
            "w2": w2,
            "b1c": b1c,
            "b2c": b2c,
            "nwb": nwb,
        })
    return maps


_NC_CACHE = []


def kernel(**inputs) -> np.ndarray:
    if not _NC_CACHE:
        _NC_CACHE.append(_build_bass())
    nc = _NC_CACHE[0]

    res = run_bass_kernel_spmd(nc, _in_maps(inputs), core_ids=list(range(N_CORES)))
    outT = np.concatenate([np.asarray(r["outT"]) for r in res.results], axis=1)
    return np.ascontiguousarray(outT.T).astype(np.float32)
